# revision 17
# baseline (speedup 1.0000x reference)
"""Trainium2 Bass kernel for CachedMultiHeadAttention.

Problem: B=16, Q=32, KV=4096, D=1024, H=16 (DH=64), fp32 in/out.
Sharding: pure data-parallel over batch — 2 batches per NeuronCore, weights
replicated, no collectives.

Per-core dataflow:
  - x^T via PE transpose; q is materialized directly as per-batch
    block-diagonal stationary operands (2 heads per [128, 64] tile) so one
    QK matmul emits scores for 2 heads at PSUM partitions 0/64.
  - K cache is loaded natural [s, D], PE-transposed, and written to SBUF as
    K^T tiles; QK runs in fp32 for accuracy (block-diag q).
  - Softmax skips max-subtraction (|scores*scale| < ~3 by construction),
    exp on ScalarE straight out of PSUM.
  - exp(scores) is PE-transposed so W@V contracts over s on partitions; W@V
    runs in float32r (single-pass fp32 matmul, 1 cycle/row at N>=256). A
    ones-column appended to V yields the softmax denominator in column 256
    of the O accumulator.
  - O is normalized (reciprocal of column 256) and PE-transposed straight
    into wv^T layout; the output projection computes y^T in float32r and
    PE-transposes back to natural [tok, D].
  - float32r matmuls obey the ISA restrictions: col_grp=0xf (output
    partition dim > 64) and even moving/output inner sizes — hence the
    transposed v/y projections (M=128) and the 258-wide W@V outputs.
"""

import numpy as np

import concourse.bass as bass
import concourse.bacc as bacc
import concourse.mybir as mybir
import concourse.tile as tile
from concourse.bass_utils import run_bass_kernel_spmd
from concourse.masks import make_identity

F32 = mybir.dt.float32
F32R = mybir.dt.float32r
BF16 = mybir.dt.bfloat16

B, Q, KV, D, H = 16, 32, 4096, 1024, 16
DH = D // H                     # 64
NCORES = 8
BL = B // NCORES                # 2 batches per core
TOK = BL * Q                    # 64 tokens per core
SCALE = float(DH) ** -0.5       # folded q*k scale (DH**-0.25 applied twice)
NSTRIPE = 8                     # stripes of 512 cached s positions
STRIPE = 512
GW = 260                        # per-quad stride in V_aug (256 V + 2 ones + 2 pad)
NWV = 258                       # W@V moving size: 256 V cols + ones col + dup ones


def _build_kernel():
    nc = bacc.Bacc(
        "TRN2",
        target_bir_lowering=False,
        debug=False,
        enable_asserts=False,
        num_devices=NCORES,
    )

    x_d = nc.dram_tensor("x", [TOK, D], F32, kind="ExternalInput").ap()
    ck_d = nc.dram_tensor("cache_k", [BL, KV, D], F32, kind="ExternalInput").ap()
    cv_d = nc.dram_tensor("cache_v", [BL, KV, D], F32R, kind="ExternalInput").ap()
    wq_d = nc.dram_tensor("Wq", [D, D], F32R, kind="ExternalInput").ap()
    wk_d = nc.dram_tensor("Wk", [D, D], F32R, kind="ExternalInput").ap()
    wv_d = nc.dram_tensor("Wv", [D, D], F32R, kind="ExternalInput").ap()
    wo_d = nc.dram_tensor("Wo", [D, D], F32R, kind="ExternalInput").ap()
    bq_d = nc.dram_tensor("bq", [D], F32, kind="ExternalInput").ap()
    bv_d = nc.dram_tensor("bv", [D], F32, kind="ExternalInput").ap()
    bo_d = nc.dram_tensor("bo", [D], F32, kind="ExternalInput").ap()
    y_d = nc.dram_tensor("y", [TOK, D], F32, kind="ExternalOutput").ap()

    with tile.TileContext(nc) as tc:
        _body(tc, x_d, ck_d, cv_d, wq_d, wk_d, wv_d, wo_d, bq_d, bv_d, bo_d, y_d)
    nc.compile()
    return nc


def _body(tc, x_d, ck_d, cv_d, wq_d, wk_d, wv_d, wo_d, bq_d, bv_d, bo_d, y_d):
    nc = tc.nc
    Exp = mybir.ActivationFunctionType.Exp

    with (
        tc.tile_pool(name="consts", bufs=1) as consts,
        tc.tile_pool(name="wo_pool", bufs=1) as wo_pool,
    ):
        identity = consts.tile([128, 128], F32)
        make_identity(nc, identity)
        ones_row = consts.tile([1, 128], F32)
        nc.vector.memset(ones_row, 1.0)

        bq_sb = consts.tile([1, D], F32)
        bv_sb = consts.tile([1, D], F32)
        bo_sb = consts.tile([1, D], F32)
        nc.sync.dma_start(out=bq_sb, in_=bq_d.rearrange("(a d) -> a d", a=1))
        nc.sync.dma_start(out=bv_sb, in_=bv_d.rearrange("(a d) -> a d", a=1))
        nc.sync.dma_start(out=bo_sb, in_=bo_d.rearrange("(a d) -> a d", a=1))

        x_sb = consts.tile([TOK, D], F32)
        nc.sync.dma_start(out=x_sb, in_=x_d)

        wo_sb = wo_pool.tile([128, 8, D], F32R)
        nc.scalar.dma_start(out=wo_sb, in_=wo_d.rearrange("(c p) d -> p c d", p=128))

        xT = consts.tile([128, 8, TOK], F32R)   # [p, k-chunk, tok]
        # block-diagonal bf16 q weights: per batch, per d-chunk [128, 64]:
        # rows 0:64 x cols 0:32 = even head, rows 64:128 x cols 32:64 = odd head
        qbd0 = consts.tile([128, 8, TOK], F32)
        qbd1 = consts.tile([128, 8, TOK], F32)
        qbd = [qbd0, qbd1]
        kT = consts.tile([128, 8, TOK], F32)    # current-token K^T
        wvT = consts.tile([128, 8, TOK], F32R)  # attention output, transposed
        vT_sb = consts.tile([128, 8, TOK], F32)
        yT_sb = consts.tile([128, 8, TOK], F32)
        v_cur0 = consts.tile([Q, 4 * GW], F32R)   # V_aug for current tokens
        v_cur1 = consts.tile([Q, 4 * GW], F32R)
        v_cur = [v_cur0, v_cur1]
        y_sb = consts.tile([TOK, D], F32)

        # ---------------- stage A: x^T and projections ----------------
        with (
            tc.tile_pool(name="w3", bufs=1) as w3,
            tc.tile_pool(name="ppsum", bufs=3, space="PSUM") as ppsum,
        ):
            wq_sb = w3.tile([128, 8, D], F32R)
            wk_sb = w3.tile([128, 8, D], F32R)
            wv_sb = w3.tile([128, 8, D], F32R)
            nc.scalar.dma_start(out=wq_sb, in_=wq_d.rearrange("(c p) d -> p c d", p=128))
            nc.scalar.dma_start(out=wk_sb, in_=wk_d.rearrange("(c p) d -> p c d", p=128))
            nc.scalar.dma_start(out=wv_sb, in_=wv_d.rearrange("(c p) d -> p c d", p=128))

            # warmup op: first PE instruction depends only on the gpsimd
            # identity, so real work never accumulates a Pool wait.
            warm_ps = ppsum.tile([128, TOK], F32, tag="pp")
            nc.tensor.matmul(
                warm_ps[0:1, 0:1], identity[:, 0:1], identity[:, 0:1],
                start=True, stop=True,
            )
            for k in range(8):
                xt_ps = ppsum.tile([128, TOK], F32, tag="pp")
                nc.tensor.matmul(
                    xt_ps, x_sb[:, 128 * k : 128 * k + 128],
                    identity[0:TOK, 0:TOK], start=True, stop=True,
                    is_transpose=True,
                )
                nc.scalar.copy(out=xT[:, k, :], in_=xt_ps)

            nc.vector.memset(qbd0, 0.0)
            nc.vector.memset(qbd1, 0.0)
            for m in range(8):
                qp = ppsum.tile([128, TOK], F32, tag="pp")
                for k in range(8):
                    nc.tensor.matmul(
                        qp,
                        wq_sb[:, k, 128 * m : 128 * m + 128],
                        xT[:, k, :],
                        start=(k == 0),
                        stop=False,
                    )
                nc.tensor.matmul(
                    qp,
                    bq_sb[0:1, 128 * m : 128 * m + 128],
                    ones_row[0:1, 0:TOK],
                    start=False,
                    stop=True,
                )
                for b in range(BL):
                    nc.scalar.copy(
                        out=qbd[b][0:64, m, 0:Q], in_=qp[0:64, Q * b : Q * b + Q]
                    )
                    nc.scalar.copy(
                        out=qbd[b][64:128, m, Q : 2 * Q],
                        in_=qp[64:128, Q * b : Q * b + Q],
                    )

            for m in range(8):
                kp = ppsum.tile([128, TOK], F32, tag="pp")
                for k in range(8):
                    nc.tensor.matmul(
                        kp,
                        wk_sb[:, k, 128 * m : 128 * m + 128],
                        xT[:, k, :],
                        start=(k == 0),
                        stop=(k == 7),
                    )
                nc.scalar.copy(out=kT[:, m, :], in_=kp)

            # v projection, transposed (M=128 keeps float32r legal), then
            # PE-transpose back to natural and scatter into V_aug layout.
            for b in range(BL):
                vags = v_cur[b].rearrange("p (g c) -> p g c", c=GW)
                nc.vector.memset(vags[:, :, 256:258].bitcast(F32), 1.0)
            for m in range(8):
                vtp = ppsum.tile([128, TOK], F32, tag="pp")
                for k in range(8):
                    nc.tensor.matmul(
                        vtp,
                        wv_sb[:, k, 128 * m : 128 * m + 128],
                        xT[:, k, :],
                        start=(k == 0),
                        stop=False,
                    )
                nc.tensor.matmul(
                    vtp,
                    bv_sb[0:1, 128 * m : 128 * m + 128],
                    ones_row[0:1, 0:TOK],
                    start=False,
                    stop=True,
                )
                nc.scalar.copy(out=vT_sb[:, m, :], in_=vtp)
            for m in range(8):
                off = GW * (m // 2) + 128 * (m % 2)
                for b in range(BL):
                    vn_ps = ppsum.tile([128, 128], F32, tag="ppn")
                    nc.tensor.matmul(
                        vn_ps[0:Q, :], vT_sb[:, m, Q * b : Q * b + Q], identity,
                        start=True, stop=True, is_transpose=True,
                    )
                    nc.scalar.copy(
                        out=v_cur[b][:, off : off + 128], in_=vn_ps[0:Q, :]
                    )

        # ---------------- main attention loop ----------------
        with (
            tc.tile_pool(name="knat", bufs=2) as knat_p,
            tc.tile_pool(name="ktp", bufs=2) as kt_p,
            tc.tile_pool(name="vaug", bufs=2) as vaug_p,
            tc.tile_pool(name="work", bufs=3) as work,
            tc.tile_pool(name="spsum", bufs=2, space="PSUM") as spsum,
            tc.tile_pool(name="trpsum", bufs=2, space="PSUM") as trpsum,
            tc.tile_pool(name="opsum", bufs=4, space="PSUM") as opsum,
        ):
            ck_r = [ck_d[b].rearrange("(j p) d -> p j d", p=128) for b in range(BL)]
            cv_r = [cv_d[b].rearrange("(j p) d -> p j d", p=128) for b in range(BL)]

            for b in range(BL):
                o_ps = []
                for g in range(4):
                    o_tile = opsum.tile([128, NWV], F32, tag="o_ps", name=f"o_b{b}g{g}")
                    o_ps.append(o_tile)

                for S in range(NSTRIPE):
                    k_nat = knat_p.tile([128, 4, D], F32)
                    nc.sync.dma_start(out=k_nat, in_=ck_r[b][:, 4 * S : 4 * S + 4, :])

                    v_aug = vaug_p.tile([128, 4, 4 * GW], F32R)
                    va4 = v_aug.rearrange("p j (g c) -> p j g c", c=GW)
                    nc.vector.memset(va4[:, :, :, 256:258].bitcast(F32), 1.0)
                    for g in range(4):
                        nc.sync.dma_start(
                            out=va4[:, :, g, 0:256],
                            in_=cv_r[b][:, 4 * S : 4 * S + 4, 256 * g : 256 * g + 256],
                        )

                    kt = kt_p.tile([128, 8, STRIPE], F32)
                    for dc in range(8):
                        tr_ps = trpsum.tile([128, STRIPE], F32, tag="tr")
                        for jj in range(4):
                            nc.tensor.matmul(
                                tr_ps[:, 128 * jj : 128 * jj + 128],
                                k_nat[:, jj, 128 * dc : 128 * dc + 128],
                                identity, start=True, stop=True,
                                is_transpose=True,
                            )
                        if dc % 2 == 0:
                            nc.scalar.copy(out=kt[:, dc, :], in_=tr_ps)
                        else:
                            nc.vector.tensor_copy(kt[:, dc, :], tr_ps)

                    for g in range(4):
                        sc_ps = spsum.tile([128, STRIPE], F32, tag="sc")
                        for half in range(2):
                            nc.tensor.matmul(
                                sc_ps[64 * half : 64 * half + 64, :],
                                qbd[b][:, 2 * g + half, :],
                                kt[:, 2 * g + half, :],
                                start=True,
                                stop=True,
                                tile_position=(0, 64 * half),
                            )
                        w_sb = work.tile([128, STRIPE], F32, tag="w_sb")
                        nc.scalar.activation(w_sb, sc_ps, Exp, scale=SCALE)

                        tr2_ps = trpsum.tile([128, STRIPE], F32, tag="tr")
                        for jj in range(4):
                            nc.tensor.matmul(
                                tr2_ps[:, 128 * jj : 128 * jj + 128],
                                w_sb[:, 128 * jj : 128 * jj + 128],
                                identity, start=True, stop=True,
                                is_transpose=True,
                            )
                        wt_sb = work.tile([128, STRIPE], F32R, tag="wt_sb")
                        nc.vector.tensor_copy(wt_sb, tr2_ps)

                        for jj in range(4):
                            nc.tensor.matmul(
                                o_ps[g],
                                wt_sb[:, 128 * jj : 128 * jj + 128],
                                v_aug[:, jj, GW * g : GW * g + NWV],
                                start=(S == 0 and jj == 0),
                                stop=False,
                                skip_group_check=True,
                            )

                # current-token tile (s = KV .. KV+Q)
                for g in range(4):
                    scur_ps = spsum.tile([128, STRIPE], F32, tag="sc")
                    for half in range(2):
                        nc.tensor.matmul(
                            scur_ps[64 * half : 64 * half + 64, 0:Q],
                            qbd[b][:, 2 * g + half, :],
                            kT[:, 2 * g + half, Q * b : Q * b + Q],
                            start=True,
                            stop=True,
                            tile_position=(0, 64 * half),
                        )
                    w_cur = work.tile([128, Q], F32, tag="w_cur")
                    nc.scalar.activation(w_cur, scur_ps[:, 0:Q], Exp, scale=SCALE)

                    trc_ps = trpsum.tile([128, STRIPE], F32, tag="tr")
                    nc.tensor.matmul(
                        trc_ps[0:Q, 0:128], w_cur, identity,
                        start=True, stop=True, is_transpose=True,
                    )
                    wt_cur = work.tile([Q, 128], F32R, tag="wt_cur")
                    nc.vector.tensor_copy(wt_cur, trc_ps[0:Q, 0:128])

                    nc.tensor.matmul(
                        o_ps[g],
                        wt_cur,
                        v_cur[b][:, GW * g : GW * g + NWV],
                        start=False,
                        stop=True,
                        skip_group_check=True,
                    )

                # normalize + extract into wv^T layout
                for g in range(4):
                    recip = work.tile([128, 1], F32, tag="recip")
                    nc.vector.reciprocal(recip, o_ps[g][:, 256:257])
                    o_sb = work.tile([128, 256], F32, tag="o_sb")
                    nc.vector.tensor_scalar_mul(o_sb, o_ps[g][:, 0:256], recip)
                    for u in range(2):
                        t_ps = trpsum.tile([128, STRIPE], F32, tag="tr")
                        nc.tensor.matmul(
                            t_ps[:, 0:128], o_sb[:, 128 * u : 128 * u + 128],
                            identity, start=True, stop=True,
                            is_transpose=True,
                        )
                        nc.vector.tensor_copy(
                            wvT[0:64, 2 * g + u, Q * b : Q * b + Q],
                            t_ps[0:64, 64 * u : 64 * u + 32],
                        )
                        nc.vector.tensor_copy(
                            wvT[64:128, 2 * g + u, Q * b : Q * b + Q],
                            t_ps[64:128, 64 * u + 32 : 64 * u + 64],
                        )

        # -------- output projection (y^T in f32r, then transpose back) -------
        with tc.tile_pool(name="ypsum", bufs=3, space="PSUM") as ypsum:
            for m in range(8):
                ytp = ypsum.tile([128, TOK], F32, tag="yt")
                for k in range(8):
                    nc.tensor.matmul(
                        ytp,
                        wo_sb[:, k, 128 * m : 128 * m + 128],
                        wvT[:, k, :],
                        start=(k == 0),
                        stop=False,
                    )
                nc.tensor.matmul(
                    ytp,
                    bo_sb[0:1, 128 * m : 128 * m + 128],
                    ones_row[0:1, 0:TOK],
                    start=False,
                    stop=True,
                )
                nc.scalar.copy(out=yT_sb[:, m, :], in_=ytp)
            for m in range(8):
                yn_ps = ypsum.tile([128, 128], F32, tag="yn")
                nc.tensor.matmul(
                    yn_ps[0:TOK, :], yT_sb[:, m, :], identity,
                    start=True, stop=True, is_transpose=True,
                )
                nc.scalar.copy(
                    out=y_sb[:, 128 * m : 128 * m + 128], in_=yn_ps[0:TOK, :]
                )
            nc.sync.dma_start(out=y_d, in_=y_sb)


_NC_CACHE = None


def _get_nc():
    global _NC_CACHE
    if _NC_CACHE is None:
        _NC_CACHE = _build_kernel()
    return _NC_CACHE


def kernel(**inputs):
    x = np.ascontiguousarray(np.asarray(inputs["x"], dtype=np.float32))
    ck = np.ascontiguousarray(np.asarray(inputs["cache_k"], dtype=np.float32))
    cv = np.ascontiguousarray(np.asarray(inputs["cache_v"], dtype=np.float32))
    weights = {
        k: np.ascontiguousarray(np.asarray(inputs[k], dtype=np.float32))
        for k in ["Wq", "Wk", "Wv", "Wo", "bq", "bv", "bo"]
    }

    nc = _get_nc()
    in_maps = []
    for c in range(NCORES):
        m = dict(weights)
        m["x"] = np.ascontiguousarray(x[c * BL : (c + 1) * BL].reshape(TOK, D))
        m["cache_k"] = np.ascontiguousarray(ck[c * BL : (c + 1) * BL])
        m["cache_v"] = np.ascontiguousarray(cv[c * BL : (c + 1) * BL])
        in_maps.append(m)

    res = run_bass_kernel_spmd(nc, in_maps, core_ids=list(range(NCORES)))
    global _LAST_RESULT
    _LAST_RESULT = res
    y = np.concatenate([r["y"].reshape(BL, Q, D) for r in res.results], axis=0)
    return y


_LAST_RESULT = None


# revision 18
# speedup vs baseline: 1.2217x; 1.2217x over previous
"""Trainium2 Bass kernel for CachedMultiHeadAttention.

Problem: B=16, Q=32, KV=4096, D=1024, H=16 (DH=64), fp32 in/out.
Sharding: pure data-parallel over batch — 2 batches per NeuronCore, weights
replicated, no collectives.

Per-core dataflow:
  - x^T via PE transpose; q is materialized directly as per-batch
    block-diagonal stationary operands (2 heads per [128, 64] tile) so one
    QK matmul emits scores for 2 heads at PSUM partitions 0/64.
  - K cache is loaded natural [s, D], PE-transposed, and written to SBUF as
    fp16 K^T tiles; QK runs in fp16 (1 cycle/row, 10 mantissa bits).
  - Softmax skips max-subtraction (|scores*scale| < ~3 by construction),
    exp on ScalarE straight out of PSUM.
  - exp(scores) is PE-transposed so W@V contracts over s on partitions; W@V
    runs in float32r (single-pass fp32 matmul, 1 cycle/row at N>=256). A
    ones-column appended to V yields the softmax denominator in column 256
    of the O accumulator.
  - O is normalized (reciprocal of column 256) and PE-transposed straight
    into wv^T layout; the output projection computes y^T in float32r and
    PE-transposes back to natural [tok, D].
  - float32r matmuls obey the ISA restrictions: col_grp=0xf (output
    partition dim > 64) and even moving/output inner sizes — hence the
    transposed v/y projections (M=128) and the 258-wide W@V outputs.
"""

import numpy as np

import concourse.bass as bass
import concourse.bacc as bacc
import concourse.mybir as mybir
import concourse.tile as tile
from concourse.bass_utils import run_bass_kernel_spmd
from concourse.masks import make_identity

F32 = mybir.dt.float32
F32R = mybir.dt.float32r
BF16 = mybir.dt.bfloat16
FP16 = mybir.dt.float16

B, Q, KV, D, H = 16, 32, 4096, 1024, 16
DH = D // H                     # 64
NCORES = 8
BL = B // NCORES                # 2 batches per core
TOK = BL * Q                    # 64 tokens per core
SCALE = float(DH) ** -0.5       # folded q*k scale (DH**-0.25 applied twice)
NSTRIPE = 8                     # stripes of 512 cached s positions
STRIPE = 512
GW = 260                        # per-quad stride in V_aug (256 V + 2 ones + 2 pad)
NWV = 258                       # W@V moving size: 256 V cols + ones col + dup ones


def _build_kernel():
    nc = bacc.Bacc(
        "TRN2",
        target_bir_lowering=False,
        debug=False,
        enable_asserts=False,
        num_devices=NCORES,
    )

    x_d = nc.dram_tensor("x", [TOK, D], F32, kind="ExternalInput").ap()
    ck_d = nc.dram_tensor("cache_k", [BL, KV, D], F32, kind="ExternalInput").ap()
    cv_d = nc.dram_tensor("cache_v", [BL, KV, D], F32R, kind="ExternalInput").ap()
    wq_d = nc.dram_tensor("Wq", [D, D], F32R, kind="ExternalInput").ap()
    wk_d = nc.dram_tensor("Wk", [D, D], F32R, kind="ExternalInput").ap()
    wv_d = nc.dram_tensor("Wv", [D, D], F32R, kind="ExternalInput").ap()
    wo_d = nc.dram_tensor("Wo", [D, D], F32R, kind="ExternalInput").ap()
    bq_d = nc.dram_tensor("bq", [D], F32, kind="ExternalInput").ap()
    bv_d = nc.dram_tensor("bv", [D], F32, kind="ExternalInput").ap()
    bo_d = nc.dram_tensor("bo", [D], F32, kind="ExternalInput").ap()
    y_d = nc.dram_tensor("y", [TOK, D], F32, kind="ExternalOutput").ap()

    with tile.TileContext(nc) as tc:
        _body(tc, x_d, ck_d, cv_d, wq_d, wk_d, wv_d, wo_d, bq_d, bv_d, bo_d, y_d)
    nc.compile()
    return nc


def _body(tc, x_d, ck_d, cv_d, wq_d, wk_d, wv_d, wo_d, bq_d, bv_d, bo_d, y_d):
    nc = tc.nc
    Exp = mybir.ActivationFunctionType.Exp

    with (
        tc.tile_pool(name="consts", bufs=1) as consts,
        tc.tile_pool(name="wo_pool", bufs=1) as wo_pool,
    ):
        identity = consts.tile([128, 128], F32)
        make_identity(nc, identity)
        ones_row = consts.tile([1, 128], F32)
        nc.vector.memset(ones_row, 1.0)

        bq_sb = consts.tile([1, D], F32)
        bv_sb = consts.tile([1, D], F32)
        bo_sb = consts.tile([1, D], F32)
        nc.sync.dma_start(out=bq_sb, in_=bq_d.rearrange("(a d) -> a d", a=1))
        nc.sync.dma_start(out=bv_sb, in_=bv_d.rearrange("(a d) -> a d", a=1))
        nc.sync.dma_start(out=bo_sb, in_=bo_d.rearrange("(a d) -> a d", a=1))

        x_sb = consts.tile([TOK, D], F32)
        nc.sync.dma_start(out=x_sb, in_=x_d)

        wo_sb = wo_pool.tile([128, 8, D], F32R)
        nc.scalar.dma_start(out=wo_sb, in_=wo_d.rearrange("(c p) d -> p c d", p=128))

        xT = consts.tile([128, 8, TOK], F32R)   # [p, k-chunk, tok]
        # block-diagonal bf16 q weights: per batch, per d-chunk [128, 64]:
        # rows 0:64 x cols 0:32 = even head, rows 64:128 x cols 32:64 = odd head
        qbd0 = consts.tile([128, 8, TOK], FP16)
        qbd1 = consts.tile([128, 8, TOK], FP16)
        qbd = [qbd0, qbd1]
        kT = consts.tile([128, 8, TOK], FP16)   # current-token K^T
        wvT = consts.tile([128, 8, TOK], F32R)  # attention output, transposed
        vT_sb = consts.tile([128, 8, TOK], F32)
        yT_sb = consts.tile([128, 8, TOK], F32)
        v_cur0 = consts.tile([Q, 4 * GW], F32R)   # V_aug for current tokens
        v_cur1 = consts.tile([Q, 4 * GW], F32R)
        v_cur = [v_cur0, v_cur1]
        y_sb = consts.tile([TOK, D], F32)

        # ---------------- stage A: x^T and projections ----------------
        with (
            tc.tile_pool(name="w3", bufs=1) as w3,
            tc.tile_pool(name="ppsum", bufs=3, space="PSUM") as ppsum,
        ):
            wq_sb = w3.tile([128, 8, D], F32R)
            wk_sb = w3.tile([128, 8, D], F32R)
            wv_sb = w3.tile([128, 8, D], F32R)
            nc.scalar.dma_start(out=wq_sb, in_=wq_d.rearrange("(c p) d -> p c d", p=128))
            nc.scalar.dma_start(out=wk_sb, in_=wk_d.rearrange("(c p) d -> p c d", p=128))
            nc.scalar.dma_start(out=wv_sb, in_=wv_d.rearrange("(c p) d -> p c d", p=128))

            # warmup op: first PE instruction depends only on the gpsimd
            # identity, so real work never accumulates a Pool wait.
            warm_ps = ppsum.tile([128, TOK], F32, tag="pp")
            nc.tensor.matmul(
                warm_ps[0:1, 0:1], identity[:, 0:1], identity[:, 0:1],
                start=True, stop=True,
            )
            for k in range(8):
                xt_ps = ppsum.tile([128, TOK], F32, tag="pp")
                nc.tensor.matmul(
                    xt_ps, x_sb[:, 128 * k : 128 * k + 128],
                    identity[0:TOK, 0:TOK], start=True, stop=True,
                    is_transpose=True,
                )
                nc.scalar.copy(out=xT[:, k, :], in_=xt_ps)

            nc.vector.memset(qbd0, 0.0)
            nc.vector.memset(qbd1, 0.0)
            for m in range(8):
                qp = ppsum.tile([128, TOK], F32, tag="pp")
                for k in range(8):
                    nc.tensor.matmul(
                        qp,
                        wq_sb[:, k, 128 * m : 128 * m + 128],
                        xT[:, k, :],
                        start=(k == 0),
                        stop=False,
                    )
                nc.tensor.matmul(
                    qp,
                    bq_sb[0:1, 128 * m : 128 * m + 128],
                    ones_row[0:1, 0:TOK],
                    start=False,
                    stop=True,
                )
                for b in range(BL):
                    nc.scalar.copy(
                        out=qbd[b][0:64, m, 0:Q], in_=qp[0:64, Q * b : Q * b + Q]
                    )
                    nc.scalar.copy(
                        out=qbd[b][64:128, m, Q : 2 * Q],
                        in_=qp[64:128, Q * b : Q * b + Q],
                    )

            for m in range(8):
                kp = ppsum.tile([128, TOK], F32, tag="pp")
                for k in range(8):
                    nc.tensor.matmul(
                        kp,
                        wk_sb[:, k, 128 * m : 128 * m + 128],
                        xT[:, k, :],
                        start=(k == 0),
                        stop=(k == 7),
                    )
                nc.scalar.copy(out=kT[:, m, :], in_=kp)

            # v projection, transposed (M=128 keeps float32r legal), then
            # PE-transpose back to natural and scatter into V_aug layout.
            for b in range(BL):
                vags = v_cur[b].rearrange("p (g c) -> p g c", c=GW)
                nc.vector.memset(vags[:, :, 256:258].bitcast(F32), 1.0)
            for m in range(8):
                vtp = ppsum.tile([128, TOK], F32, tag="pp")
                for k in range(8):
                    nc.tensor.matmul(
                        vtp,
                        wv_sb[:, k, 128 * m : 128 * m + 128],
                        xT[:, k, :],
                        start=(k == 0),
                        stop=False,
                    )
                nc.tensor.matmul(
                    vtp,
                    bv_sb[0:1, 128 * m : 128 * m + 128],
                    ones_row[0:1, 0:TOK],
                    start=False,
                    stop=True,
                )
                nc.scalar.copy(out=vT_sb[:, m, :], in_=vtp)
            for m in range(8):
                off = GW * (m // 2) + 128 * (m % 2)
                for b in range(BL):
                    vn_ps = ppsum.tile([128, 128], F32, tag="ppn")
                    nc.tensor.matmul(
                        vn_ps[0:Q, :], vT_sb[:, m, Q * b : Q * b + Q], identity,
                        start=True, stop=True, is_transpose=True,
                    )
                    nc.scalar.copy(
                        out=v_cur[b][:, off : off + 128], in_=vn_ps[0:Q, :]
                    )

        # ---------------- main attention loop ----------------
        with (
            tc.tile_pool(name="knat", bufs=2) as knat_p,
            tc.tile_pool(name="ktp", bufs=2) as kt_p,
            tc.tile_pool(name="vaug", bufs=2) as vaug_p,
            tc.tile_pool(name="work", bufs=3) as work,
            tc.tile_pool(name="spsum", bufs=2, space="PSUM") as spsum,
            tc.tile_pool(name="trpsum", bufs=2, space="PSUM") as trpsum,
            tc.tile_pool(name="opsum", bufs=4, space="PSUM") as opsum,
        ):
            ck_r = [ck_d[b].rearrange("(j p) d -> p j d", p=128) for b in range(BL)]
            cv_r = [cv_d[b].rearrange("(j p) d -> p j d", p=128) for b in range(BL)]

            for b in range(BL):
                o_ps = []
                for g in range(4):
                    o_tile = opsum.tile([128, NWV], F32, tag="o_ps", name=f"o_b{b}g{g}")
                    o_ps.append(o_tile)

                for S in range(NSTRIPE):
                    k_nat = knat_p.tile([128, 4, D], F32)
                    nc.sync.dma_start(out=k_nat, in_=ck_r[b][:, 4 * S : 4 * S + 4, :])

                    v_aug = vaug_p.tile([128, 4, 4 * GW], F32R)
                    va4 = v_aug.rearrange("p j (g c) -> p j g c", c=GW)
                    nc.vector.memset(va4[:, :, :, 256:258].bitcast(F32), 1.0)
                    for g in range(4):
                        nc.sync.dma_start(
                            out=va4[:, :, g, 0:256],
                            in_=cv_r[b][:, 4 * S : 4 * S + 4, 256 * g : 256 * g + 256],
                        )

                    kt = kt_p.tile([128, 8, STRIPE], FP16)
                    for dc in range(8):
                        tr_ps = trpsum.tile([128, STRIPE], F32, tag="tr")
                        for jj in range(4):
                            nc.tensor.matmul(
                                tr_ps[:, 128 * jj : 128 * jj + 128],
                                k_nat[:, jj, 128 * dc : 128 * dc + 128],
                                identity, start=True, stop=True,
                                is_transpose=True,
                            )
                        if dc % 2 == 0:
                            nc.scalar.copy(out=kt[:, dc, :], in_=tr_ps)
                        else:
                            nc.vector.tensor_copy(kt[:, dc, :], tr_ps)

                    for g in range(4):
                        sc_ps = spsum.tile([128, STRIPE], F32, tag="sc")
                        for half in range(2):
                            nc.tensor.matmul(
                                sc_ps[64 * half : 64 * half + 64, :],
                                qbd[b][:, 2 * g + half, :],
                                kt[:, 2 * g + half, :],
                                start=True,
                                stop=True,
                                tile_position=(0, 64 * half),
                            )
                        w_sb = work.tile([128, STRIPE], F32, tag="w_sb")
                        nc.scalar.activation(w_sb, sc_ps, Exp, scale=SCALE)

                        tr2_ps = trpsum.tile([128, STRIPE], F32, tag="tr")
                        for jj in range(4):
                            nc.tensor.matmul(
                                tr2_ps[:, 128 * jj : 128 * jj + 128],
                                w_sb[:, 128 * jj : 128 * jj + 128],
                                identity, start=True, stop=True,
                                is_transpose=True,
                            )
                        wt_sb = work.tile([128, STRIPE], F32R, tag="wt_sb")
                        nc.vector.tensor_copy(wt_sb, tr2_ps)

                        for jj in range(4):
                            nc.tensor.matmul(
                                o_ps[g],
                                wt_sb[:, 128 * jj : 128 * jj + 128],
                                v_aug[:, jj, GW * g : GW * g + NWV],
                                start=(S == 0 and jj == 0),
                                stop=False,
                                skip_group_check=True,
                            )

                # current-token tile (s = KV .. KV+Q)
                for g in range(4):
                    scur_ps = spsum.tile([128, STRIPE], F32, tag="sc")
                    for half in range(2):
                        nc.tensor.matmul(
                            scur_ps[64 * half : 64 * half + 64, 0:Q],
                            qbd[b][:, 2 * g + half, :],
                            kT[:, 2 * g + half, Q * b : Q * b + Q],
                            start=True,
                            stop=True,
                            tile_position=(0, 64 * half),
                        )
                    w_cur = work.tile([128, Q], F32, tag="w_cur")
                    nc.scalar.activation(w_cur, scur_ps[:, 0:Q], Exp, scale=SCALE)

                    trc_ps = trpsum.tile([128, STRIPE], F32, tag="tr")
                    nc.tensor.matmul(
                        trc_ps[0:Q, 0:128], w_cur, identity,
                        start=True, stop=True, is_transpose=True,
                    )
                    wt_cur = work.tile([Q, 128], F32R, tag="wt_cur")
                    nc.vector.tensor_copy(wt_cur, trc_ps[0:Q, 0:128])

                    nc.tensor.matmul(
                        o_ps[g],
                        wt_cur,
                        v_cur[b][:, GW * g : GW * g + NWV],
                        start=False,
                        stop=True,
                        skip_group_check=True,
                    )

                # normalize + extract into wv^T layout
                for g in range(4):
                    recip = work.tile([128, 1], F32, tag="recip")
                    nc.vector.reciprocal(recip, o_ps[g][:, 256:257])
                    o_sb = work.tile([128, 256], F32, tag="o_sb")
                    nc.vector.tensor_scalar_mul(o_sb, o_ps[g][:, 0:256], recip)
                    for u in range(2):
                        t_ps = trpsum.tile([128, STRIPE], F32, tag="tr")
                        nc.tensor.matmul(
                            t_ps[:, 0:128], o_sb[:, 128 * u : 128 * u + 128],
                            identity, start=True, stop=True,
                            is_transpose=True,
                        )
                        nc.vector.tensor_copy(
                            wvT[0:64, 2 * g + u, Q * b : Q * b + Q],
                            t_ps[0:64, 64 * u : 64 * u + 32],
                        )
                        nc.vector.tensor_copy(
                            wvT[64:128, 2 * g + u, Q * b : Q * b + Q],
                            t_ps[64:128, 64 * u + 32 : 64 * u + 64],
                        )

        # -------- output projection (y^T in f32r, then transpose back) -------
        with tc.tile_pool(name="ypsum", bufs=3, space="PSUM") as ypsum:
            for m in range(8):
                ytp = ypsum.tile([128, TOK], F32, tag="yt")
                for k in range(8):
                    nc.tensor.matmul(
                        ytp,
                        wo_sb[:, k, 128 * m : 128 * m + 128],
                        wvT[:, k, :],
                        start=(k == 0),
                        stop=False,
                    )
                nc.tensor.matmul(
                    ytp,
                    bo_sb[0:1, 128 * m : 128 * m + 128],
                    ones_row[0:1, 0:TOK],
                    start=False,
                    stop=True,
                )
                nc.scalar.copy(out=yT_sb[:, m, :], in_=ytp)
            for m in range(8):
                yn_ps = ypsum.tile([128, 128], F32, tag="yn")
                nc.tensor.matmul(
                    yn_ps[0:TOK, :], yT_sb[:, m, :], identity,
                    start=True, stop=True, is_transpose=True,
                )
                nc.scalar.copy(
                    out=y_sb[:, 128 * m : 128 * m + 128], in_=yn_ps[0:TOK, :]
                )
            nc.sync.dma_start(out=y_d, in_=y_sb)


_NC_CACHE = None


def _get_nc():
    global _NC_CACHE
    if _NC_CACHE is None:
        _NC_CACHE = _build_kernel()
    return _NC_CACHE


def kernel(**inputs):
    x = np.ascontiguousarray(np.asarray(inputs["x"], dtype=np.float32))
    ck = np.ascontiguousarray(np.asarray(inputs["cache_k"], dtype=np.float32))
    cv = np.ascontiguousarray(np.asarray(inputs["cache_v"], dtype=np.float32))
    weights = {
        k: np.ascontiguousarray(np.asarray(inputs[k], dtype=np.float32))
        for k in ["Wq", "Wk", "Wv", "Wo", "bq", "bv", "bo"]
    }

    nc = _get_nc()
    in_maps = []
    for c in range(NCORES):
        m = dict(weights)
        m["x"] = np.ascontiguousarray(x[c * BL : (c + 1) * BL].reshape(TOK, D))
        m["cache_k"] = np.ascontiguousarray(ck[c * BL : (c + 1) * BL])
        m["cache_v"] = np.ascontiguousarray(cv[c * BL : (c + 1) * BL])
        in_maps.append(m)

    res = run_bass_kernel_spmd(nc, in_maps, core_ids=list(range(NCORES)))
    global _LAST_RESULT
    _LAST_RESULT = res
    y = np.concatenate([r["y"].reshape(BL, Q, D) for r in res.results], axis=0)
    return y


_LAST_RESULT = None
